# revision 34
# baseline (speedup 1.0000x reference)
"""GQA decode attention kernel for Trainium2, tensor-parallel over 8 kv heads.

Contract: kernel(**inputs) takes FULL inputs (numpy), returns FULL output.
Shapes are hardcoded: x[8,16,4096], w_in[6144,4096], w_out[4096,4096],
k_cache[8,4096,8,128], v_cache[8,4096,8,128], offset=4080.

Per-core (core g owns kv head g, q heads 4g..4g+3):
  qkv = x @ w_in_g.T            -> [128, 768] (q 512 | k 128 | v 128)
  rope(q, k), scatter new k/v into cache tail (T=4096)
  scoresT[t, (r,s)] = kkT chunks.T @ qT    (PE, per batch)
  expS = exp(scores)            (ACT, no max-sub: |scores| < ~8)
  denom = ones.T @ expS         (4 wide accumulating MMs + DVE tree)
  outT = vv.T @ expS            (PE accumulate) ; scaled by 1/denom
  partial = attn.T stationary vs w_out column slices -> [128, 4096]
Host sums the 8 partials.

v2 schedule: PE warm-up burst, w_in split in 4 slice tiles (fine-grained
DMA deps), kv stream gated behind w_in via dummy DMA dep, w_out streamed
late as 8 column-slice tiles, full-batch out projection.
"""

import os
import sys

for _p in ("/opt/trn_rl_repo", "/root/.axon_site/_ro/trn_rl_repo"):
    if os.path.isdir(_p) and _p not in sys.path:
        sys.path.insert(0, _p)

import numpy as np
import ml_dtypes

BF16 = ml_dtypes.bfloat16

B, S, E = 8, 16, 4096
HQ, HKV, HD = 32, 8, 128
R = HQ // HKV          # 4 q heads per kv head
T = 4096               # cache length == offset + S
OFFSET = 4080
NCORES = 8
ROPE_BASE = 10000.0
BS = B * S             # 128 rows
QF = R * HD            # 512 q features per core
KCH = E // 128         # 32 contraction chunks for qkv proj
TCH = T // 128         # 32 T chunks
NSL = 8                # w_in DMA slices
KPS = KCH // NSL       # k-chunks per slice

_CACHED = {}


def _build_program():
    """Build the Bass program once (same program for all cores)."""
    from concourse import bacc, bass, masks, mybir
    from concourse import tile

    f32 = mybir.dt.float32
    bf16 = mybir.dt.bfloat16
    f16 = mybir.dt.float16
    ActExp = mybir.ActivationFunctionType.Exp

    nc = bacc.Bacc(
        "TRN2",
        target_bir_lowering=False,
        debug=False,
        enable_asserts=False,
        num_devices=NCORES,
    )

    # DRAM I/O (per-core shards, host pre-permuted so every DMA is a plain
    # [128, N] contiguous-per-partition transfer)
    xT_d = nc.dram_tensor("xT", [128, KCH * 128], bf16, kind="ExternalInput").ap()
    win_d = nc.dram_tensor(
        "w_inT", [NSL, 128, KPS * 768], bf16, kind="ExternalInput"
    ).ap()
    wout_d = nc.dram_tensor(
        "w_outT", [8, 128, R * 512], bf16, kind="ExternalInput"
    ).ap()
    ropec_d = nc.dram_tensor("rope_c", [128, 640], f32, kind="ExternalInput").ap()
    ropes_d = nc.dram_tensor("rope_s", [128, 640], f32, kind="ExternalInput").ap()
    kT_d = nc.dram_tensor("kT", [B, 128, T], bf16, kind="ExternalInput").ap()
    v_d = nc.dram_tensor("vperm", [B, 128, T], bf16, kind="ExternalInput").ap()
    out_d = nc.dram_tensor("out", [BS, E], f16, kind="ExternalOutput").ap()

    with tile.TileContext(nc) as tc:
        from contextlib import ExitStack

        with ExitStack() as ctx:
            const = ctx.enter_context(tc.tile_pool(name="const", bufs=1))
            woutp = ctx.enter_context(tc.tile_pool(name="woutp", bufs=1))
            work = ctx.enter_context(tc.tile_pool(name="work", bufs=1))
            # kst: one rotation shared by the 4 w_in slices then the 8 kk
            # tiles -- w_in buffers are dead after the qkv projection, so
            # kk2..5 reuse them and every kk DMA can issue by ~25us (the
            # issue is then ring-paced, not PE-paced)
            kst = ctx.enter_context(tc.tile_pool(name="kst", bufs=8))
            vst = ctx.enter_context(tc.tile_pool(name="vst", bufs=6))
            epool = ctx.enter_context(tc.tile_pool(name="epool", bufs=2))
            opool = ctx.enter_context(tc.tile_pool(name="opool", bufs=3))
            # PSUM budget (8 banks): ps_sc 4x[128,512]=4 (scores rotate 4
            # quarter-banks so QK never waits on ACT; also transposes, bc,
            # warm-up, out-projection), ps_big 1x[128,1024]=2 (qkv proj),
            # ps_out 1 ([128,128]: two 64-col PV accumulators packed),
            # ps_den 1 ([1,512] denominator)
            ps_sc = ctx.enter_context(tc.tile_pool(name="ps_sc", bufs=4, space="PSUM"))
            ps_big = ctx.enter_context(
                tc.tile_pool(name="ps_big", bufs=1, space="PSUM")
            )
            ps_out = ctx.enter_context(
                tc.tile_pool(name="ps_out", bufs=1, space="PSUM")
            )
            ps_den = ctx.enter_context(
                tc.tile_pool(name="ps_den", bufs=1, space="PSUM")
            )

            # ---- constants
            ident = const.tile([128, 128], f32, tag="ident")
            masks.make_identity(nc, ident[:])
            ones_col = const.tile([128, 1], bf16, tag="ones_col")
            nc.vector.memset(ones_col[:], 1.0)
            ones_row = const.tile([1, 128], f32, tag="ones_row")
            nc.vector.memset(ones_row[:], 1.0)

            # ---- PE warm-up: ~3.5us of junk matmuls so HAM reaches K=8/8
            # before the first real matmul (w_in slice 0 lands ~6us in).
            wstat = const.tile([128, 128], bf16, tag="wstat")
            nc.vector.memset(wstat[:], 0.0)
            wmov = const.tile([128, 512], bf16, tag="wmov")
            nc.vector.memset(wmov[:], 0.0)
            warm_ps = ps_sc.tile([128, 512], f32, tag="sc", name="warm")
            for _ in range(40):
                nc.tensor.matmul(
                    warm_ps[:, 0:256], wstat[:], wmov[:, 0:256], start=True, stop=True
                )

            # ---- weight / activation loads. Both HWDGE queues (scalar +
            # sync) are kept loaded concurrently: one queue alone sustains
            # only ~330 GB/s, both together ~425 GB/s.
            xT = const.tile([128, KCH * 128], bf16, tag="xT")
            w_inT = [
                kst.tile([128, KPS * 768], bf16, tag="kst", name=f"w_inT{s}")
                for s in range(NSL)
            ]
            ropeC = const.tile([128, 640], f32, tag="ropeC")
            ropeS = const.tile([128, 640], f32, tag="ropeS")
            nc.scalar.dma_start(xT[:, 0 : 16 * 128], xT_d[:, 0 : 16 * 128])
            nc.sync.dma_start(xT[:, 16 * 128 :], xT_d[:, 16 * 128 :])
            for s in range(NSL):
                eng = nc.scalar if s % 2 == 0 else nc.sync
                eng.dma_start(w_inT[s][:], win_d[s][:])
            nc.scalar.dma_start(ropeC[:], ropec_d[:])
            nc.scalar.dma_start(ropeS[:], ropes_d[:])
            # w_out column-slice tiles; DMAs issued late (inside batch loop)
            w_outT = [
                woutp.tile([128, R * 512], bf16, tag=f"w_outT{n}", name=f"w_outT{n}")
                for n in range(8)
            ]

            # ---- phase 1: qkv projection [128, 768] in psum
            qkv_ps = ps_big.tile([128, 1024], f32, tag="big", name="qkv")
            for s in range(NSL):
                for j in range(KPS):
                    k = s * KPS + j
                    nc.tensor.matmul(
                        qkv_ps[:, 0:512],
                        xT[:, k * 128 : (k + 1) * 128],
                        w_inT[s][:, j * 768 : j * 768 + 512],
                        start=(k == 0),
                        stop=(k == KCH - 1),
                    )
                    nc.tensor.matmul(
                        qkv_ps[:, 512:768],
                        xT[:, k * 128 : (k + 1) * 128],
                        w_inT[s][:, j * 768 + 512 : (j + 1) * 768],
                        start=(k == 0),
                        stop=(k == KCH - 1),
                    )

            # v block to sbuf (bf16), used to patch the v cache tail per batch
            # (patch is an SBUF->SBUF DMA: compute engines need quadrant-
            # aligned partition starts, DMA does not)
            v_sb = work.tile([128, 128], bf16, tag="v_sb")
            nc.vector.tensor_copy(v_sb[:], qkv_ps[:, 640:768])

            # ---- rope on q|k (cols 0:640): out = t*C + rot(t)*S
            rot = work.tile([128, 640], f32, tag="rot")
            rot4 = rot[:].rearrange("p (blk h j) -> p blk h j", blk=5, h=2)
            ps4 = qkv_ps[:, 0:640].rearrange("p (blk h j) -> p blk h j", blk=5, h=2)
            nc.vector.tensor_copy(rot4[:, :, 0, :], ps4[:, :, 1, :])
            nc.vector.tensor_copy(rot4[:, :, 1, :], ps4[:, :, 0, :])
            # stage rope tables via copies: TT ops can't carry a DMA-queue
            # semaphore wait, so the copies absorb the DMA dependency
            ropeCs = work.tile([128, 640], f32, tag="ropeCs")
            nc.vector.tensor_copy(ropeCs[:], ropeC[:])
            ropeSs = work.tile([128, 640], f32, tag="ropeSs")
            nc.vector.tensor_copy(ropeSs[:], ropeS[:])
            roped = work.tile([128, 640], f32, tag="roped")
            nc.vector.tensor_mul(roped[:], qkv_ps[:, 0:640], ropeCs[:])
            t2 = work.tile([128, 640], f32, tag="t2")
            nc.vector.tensor_mul(t2[:], rot[:], ropeSs[:])
            nc.vector.tensor_add(roped[:], roped[:], t2[:])

            # ---- transpose q heads -> q_allT [128(d), (b r s)] bf16
            q_allT = work.tile([128, B * R * S], bf16, tag="q_allT")
            qv = q_allT[:].rearrange("p (b r s) -> p b r s", b=B, r=R)
            for r in range(R):
                tp = ps_sc.tile([128, 512], f32, tag="sc")
                nc.tensor.transpose(
                    tp[:, 0:128], roped[:, r * 128 : (r + 1) * 128], ident[:]
                )
                nc.vector.tensor_copy(
                    qv[:, :, r, :], tp[:, 0:128].rearrange("p (b s) -> p b s", b=B)
                )
            # transpose new k -> kT_sb [128(d), (b s)] bf16
            kT_sb = work.tile([128, 128], bf16, tag="kT_sb")
            tpk = ps_sc.tile([128, 512], f32, tag="sc")
            nc.tensor.transpose(tpk[:, 0:128], roped[:, 512:640], ident[:])
            nc.vector.tensor_copy(kT_sb[:], tpk[:, 0:128])

            # attention outputs per head, [128(d), (b s)] bf16, all batches
            attn_allT = [
                work.tile([128, BS], bf16, tag=f"attn{r}", name=f"attn{r}")
                for r in range(R)
            ]

            # ---- phase 2: attention per batch
            pending = []  # (b, outT_ps, recip) awaiting bc+scale

            # single psum bank holds both in-flight PV accumulators (64 cols
            # each, alternating); start=True clears has_written bits bank-wide
            # but not data, and the neighbor group is always stopped by then
            outT_duo = ps_out.tile([128, 128], f32, tag="po", name="outT_duo")

            def flush_pending():
                while pending:
                    pb, p_recip = pending.pop(0)
                    lo = (pb % 2) * 64
                    bc_ps = ps_sc.tile([128, 512], f32, tag="sc", name="bc_ps")
                    nc.tensor.matmul(
                        bc_ps[:, 0:64], ones_row[:], p_recip[:], start=True, stop=True
                    )
                    bc_sb = epool.tile([128, 64], f32, tag="bc_sb", name="bc_sb")
                    nc.vector.tensor_copy(bc_sb[:], bc_ps[:, 0:64])
                    for r in range(R):
                        nc.vector.tensor_mul(
                            attn_allT[r][:, pb * S : (pb + 1) * S],
                            outT_duo[:, lo + r * S : lo + (r + 1) * S],
                            bc_sb[:, r * S : (r + 1) * S],
                        )

            # kk stream: all 8 issued up front on the sync queue (the sync
            # engine has no compute, so blocking on the kst buffer semaphore
            # for kk6/kk7 is harmless); ring order = consumption order
            kk_tiles = {}
            for pb in range(B):
                kk = kst.tile([128, T], bf16, tag="kst", name=f"kk{pb}")
                nc.sync.dma_start(kk[:], kT_d[pb][:])
                kk_tiles[pb] = kk
            # vv stream on the scalar queue: vv0..5 up front (6 bufs); vv6/7
            # issued inside the batch loop (their buffer wait is PV(b0)/PV(b1),
            # which would deadlock the activation stream if issued here)
            vv_tiles = {}
            for pb in range(6):
                vv = vst.tile([128, T], bf16, tag="vst", name=f"vv{pb}")
                nc.scalar.dma_start(vv[:], v_d[pb][:])
                vv_tiles[pb] = vv

            for b in range(B):
                kk = kk_tiles.pop(b)
                # overwrite stale tail keys with roped new keys (gpsimd DMA:
                # a vector copy here would stall the DVE stream on kv data)
                nc.gpsimd.dma_start(
                    kk[:, OFFSET:T], kT_sb[:, b * S : (b + 1) * S]
                )
                vv = vv_tiles.pop(b)
                # patch new v tokens: chunk 31, partitions 112..127
                nc.gpsimd.dma_start(
                    vv[112:128, 31 * 128 : 32 * 128], v_sb[b * S : (b + 1) * S, :]
                )

                expS = epool.tile([128, TCH * 64], bf16, tag="expS")
                denom_ps = ps_den.tile([1, 512], f32, tag="den")
                olo = (b % 2) * 64
                # QK into 4 rotating quarter-banks; exp chases per quarter
                for q in range(4):
                    sc = ps_sc.tile([128, 512], f32, tag="sc")
                    for tt in range(8):
                        t = q * 8 + tt
                        nc.tensor.matmul(
                            sc[:, tt * 64 : (tt + 1) * 64],
                            kk[:, t * 128 : (t + 1) * 128],
                            q_allT[:, b * 64 : (b + 1) * 64],
                            start=True,
                            stop=True,
                        )
                    nc.scalar.activation(
                        expS[:, q * 512 : (q + 1) * 512], sc[:], ActExp
                    )
                    if q == 1:
                        # vv6/vv7: issued after this batch's first exps so
                        # their buffer wait (PV(b0)/PV(b1)) is satisfied and
                        # the activation stream doesn't stall
                        if b < 2:
                            nvv = vst.tile(
                                [128, T], bf16, tag="vst", name=f"vv{b + 6}"
                            )
                            nc.scalar.dma_start(nvv[:], v_d[b + 6][:])
                            vv_tiles[b + 6] = nvv
                        # w_out: gate each slice's DMA behind a tiny memset
                        # (WAW dep) so the scheduler cannot hoist them ahead
                        # of the w_in/kv streams on either ring; 4+4 across
                        # both queues, landing right behind the kv data
                        if b == 2:
                            for n in range(8):
                                nc.vector.memset(w_outT[n][0:1, 0:1], 0.0)
                            for n in range(4):
                                nc.sync.dma_start(w_outT[n][:], wout_d[n][:])
                            for n in range(4, 8):
                                nc.scalar.dma_start(w_outT[n][:], wout_d[n][:])
                # PV accumulation over all 32 chunks
                for t in range(TCH):
                    nc.tensor.matmul(
                        outT_duo[:, olo : olo + 64],
                        vv[:, t * 128 : (t + 1) * 128],
                        expS[:, t * 64 : (t + 1) * 64],
                        start=(t == 0),
                        stop=(t == TCH - 1),
                    )
                # denominator: four wide N=512 matmuls, one accumulation
                # group -> [1, 512] partial sums
                for i in range(4):
                    nc.tensor.matmul(
                        denom_ps[:],
                        ones_col[:],
                        expS[:, i * 512 : (i + 1) * 512],
                        start=(i == 0),
                        stop=(i == 3),
                    )
                # normalize the previous batch (its reciprocal is long ready)
                flush_pending()
                # tree-reduce the 8 chunk-groups: [1,512] -> [1,64]
                # (TT ops can read at most one PSUM input, so stage to SBUF)
                dred = epool.tile([1, 512], f32, tag="dred", name="dred")
                nc.vector.tensor_copy(dred[:, 0:512], denom_ps[:])
                nc.vector.tensor_add(dred[:, 0:256], dred[:, 0:256], dred[:, 256:512])
                nc.vector.tensor_add(dred[:, 0:128], dred[:, 0:128], dred[:, 128:256])
                nc.vector.tensor_add(dred[:, 0:64], dred[:, 0:64], dred[:, 64:128])
                recip = epool.tile([1, 64], f32, tag="recip", name="recip")
                nc.vector.reciprocal(recip[:], dred[:, 0:64])
                pending.append((b, recip))
            flush_pending()

            # ---- phase 3: out projection, full batch [128, 4096]
            # out[bs, e_out] = sum_r attn_rT.T @ w_out[r-block, e_out-slice]
            ActCopy = mybir.ActivationFunctionType.Copy
            for n in range(8):
                po = ps_sc.tile([128, 512], f32, tag="sc")
                for r in range(R):
                    nc.tensor.matmul(
                        po[:],
                        attn_allT[r][:],
                        w_outT[n][:, r * 512 : (r + 1) * 512],
                        start=(r == 0),
                        stop=(r == R - 1),
                    )
                osb = opool.tile([128, 512], f16, tag="osb")
                # alternate the psum->sbuf drain between DVE and ACT so
                # consecutive slices overlap
                if n % 2 == 0:
                    nc.vector.tensor_copy(osb[:], po[:])
                else:
                    nc.scalar.activation(osb[:], po[:], ActCopy)
                nc.sync.dma_start(out_d[:, n * 512 : (n + 1) * 512], osb[:])

    nc.compile()
    return nc


def _host_shards(x, w_in, w_out, k_cache, v_cache):
    """Per-core input dicts, pre-permuted for contiguous [128, N] DMAs."""
    x2 = np.ascontiguousarray(x.reshape(BS, E))
    xT_perm = (
        x2.T.reshape(KCH, 128, 128).transpose(1, 0, 2).reshape(128, KCH * 128)
    ).astype(BF16)

    # rope tables (identical on all cores); fold attn scale into q blocks
    inv_freq = 1.0 / (ROPE_BASE ** (np.arange(0, HD, 2, dtype=np.float64) / HD))
    pos = (OFFSET + np.arange(S)).astype(np.float64)
    ang = pos[:, None] * inv_freq[None, :]          # [S, 64]
    cos16 = np.cos(ang).astype(np.float32)
    sin16 = np.sin(ang).astype(np.float32)
    scale = np.float32(1.0 / np.sqrt(HD))
    C = np.zeros((128, 640), np.float32)
    Sn = np.zeros((128, 640), np.float32)
    srow = np.arange(128) % S                        # partition p=(b,s) -> s
    for blk in range(5):
        blk_scale = scale if blk < 4 else np.float32(1.0)
        C[:, blk * 128 : blk * 128 + 64] = cos16[srow] * blk_scale
        C[:, blk * 128 + 64 : blk * 128 + 128] = cos16[srow] * blk_scale
        Sn[:, blk * 128 : blk * 128 + 64] = -sin16[srow] * blk_scale
        Sn[:, blk * 128 + 64 : blk * 128 + 128] = sin16[srow] * blk_scale

    shards = []
    for g in range(NCORES):
        rows = np.concatenate(
            [
                w_in[QF * g : QF * (g + 1)],
                w_in[E + HD * g : E + HD * (g + 1)],
                w_in[E + HKV * HD + HD * g : E + HKV * HD + HD * (g + 1)],
            ],
            axis=0,
        )  # [768, 4096]
        w_inT_perm = (
            rows.T.reshape(KCH, 128, 768)
            .transpose(1, 0, 2)
            .reshape(128, KCH * 768)
        ).astype(BF16)
        w_inT_sl = np.ascontiguousarray(
            w_inT_perm.reshape(128, NSL, KPS * 768).transpose(1, 0, 2)
        )  # [NSL, 128, KPS*768]
        w_outT = np.ascontiguousarray(
            w_out[:, QF * g : QF * (g + 1)].T
        )  # [512 e_in, 4096 e_out]
        wout8 = np.empty((8, 128, R * 512), np.float32)
        for n in range(8):
            for r in range(R):
                wout8[n, :, r * 512 : (r + 1) * 512] = w_outT[
                    r * 128 : (r + 1) * 128, n * 512 : (n + 1) * 512
                ]
        kT = np.ascontiguousarray(
            k_cache[:, :, g, :].transpose(0, 2, 1)
        ).astype(BF16)  # [B, 128(d), T]
        vperm = np.ascontiguousarray(
            v_cache[:, :, g, :]
            .reshape(B, TCH, 128, HD)
            .transpose(0, 2, 1, 3)
            .reshape(B, 128, T)
        ).astype(BF16)  # [B, 128(t_in), (chunk d)]
        shards.append(
            {
                "xT": xT_perm,
                "w_inT": w_inT_sl,
                "w_outT": wout8.astype(BF16),
                "rope_c": C,
                "rope_s": Sn,
                "kT": kT,
                "vperm": vperm,
            }
        )
    return shards


def _get_nc():
    if "nc" not in _CACHED:
        _CACHED["nc"] = _build_program()
    return _CACHED["nc"]


def run_on_hw(in_maps, trace=False, **kw):
    from concourse import bass_utils

    nc = _get_nc()
    return bass_utils.run_bass_kernel_spmd(
        nc, in_maps, core_ids=list(range(NCORES)), trace=trace, **kw
    )


def kernel(x, w_in, w_out, k_cache, v_cache, offset):
    assert int(offset) == OFFSET and x.shape == (B, S, E)
    shards = _host_shards(
        np.asarray(x, np.float32),
        np.asarray(w_in, np.float32),
        np.asarray(w_out, np.float32),
        np.asarray(k_cache, np.float32),
        np.asarray(v_cache, np.float32),
    )
    res = run_on_hw(shards)
    out = np.zeros((BS, E), np.float64)
    for g in range(NCORES):
        out += np.asarray(res.results[g]["out"], np.float64)
    return out.astype(np.float32).reshape(B, S, E)


# revision 35
# speedup vs baseline: 1.0810x; 1.0810x over previous
"""GQA decode attention kernel for Trainium2, tensor-parallel over 8 kv heads.

Contract: kernel(**inputs) takes FULL inputs (numpy), returns FULL output.
Shapes are hardcoded: x[8,16,4096], w_in[6144,4096], w_out[4096,4096],
k_cache[8,4096,8,128], v_cache[8,4096,8,128], offset=4080.

Per-core (core g owns kv head g, q heads 4g..4g+3):
  qkv = x @ w_in_g.T            -> [128, 768] (q 512 | k 128 | v 128)
  rope(q, k), scatter new k/v into cache tail (T=4096)
  scoresT[t, (r,s)] = kkT chunks.T @ qT    (PE, per batch)
  expS = exp(scores)            (ACT, no max-sub: |scores| < ~8)
  denom = ones.T @ expS         (4 wide accumulating MMs + DVE tree)
  outT = vv.T @ expS            (PE accumulate) ; scaled by 1/denom
  partial = attn.T stationary vs w_out column slices -> [128, 4096]
Host sums the 8 partials.

v2 schedule: PE warm-up burst, w_in split in 4 slice tiles (fine-grained
DMA deps), kv stream gated behind w_in via dummy DMA dep, w_out streamed
late as 8 column-slice tiles, full-batch out projection.
"""

import os
import sys

for _p in ("/opt/trn_rl_repo", "/root/.axon_site/_ro/trn_rl_repo"):
    if os.path.isdir(_p) and _p not in sys.path:
        sys.path.insert(0, _p)

import numpy as np
import ml_dtypes

BF16 = ml_dtypes.bfloat16

B, S, E = 8, 16, 4096
HQ, HKV, HD = 32, 8, 128
R = HQ // HKV          # 4 q heads per kv head
T = 4096               # cache length == offset + S
OFFSET = 4080
NCORES = 8
ROPE_BASE = 10000.0
BS = B * S             # 128 rows
QF = R * HD            # 512 q features per core
KCH = E // 128         # 32 contraction chunks for qkv proj
TCH = T // 128         # 32 T chunks
NSL = 8                # w_in DMA slices
KPS = KCH // NSL       # k-chunks per slice

_CACHED = {}


def _build_program():
    """Build the Bass program once (same program for all cores)."""
    from concourse import bacc, bass, masks, mybir
    from concourse import tile

    f32 = mybir.dt.float32
    bf16 = mybir.dt.bfloat16
    f16 = mybir.dt.float16
    ActExp = mybir.ActivationFunctionType.Exp

    nc = bacc.Bacc(
        "TRN2",
        target_bir_lowering=False,
        debug=False,
        enable_asserts=False,
        num_devices=NCORES,
    )

    # DRAM I/O (per-core shards, host pre-permuted so every DMA is a plain
    # [128, N] contiguous-per-partition transfer)
    xT_d = nc.dram_tensor("xT", [128, KCH * 128], bf16, kind="ExternalInput").ap()
    win_d = nc.dram_tensor(
        "w_inT", [NSL, 128, KPS * 768], bf16, kind="ExternalInput"
    ).ap()
    wout_d = nc.dram_tensor(
        "w_outT", [8, 128, R * 512], bf16, kind="ExternalInput"
    ).ap()
    ropec_d = nc.dram_tensor("rope_c", [128, 640], f32, kind="ExternalInput").ap()
    ropes_d = nc.dram_tensor("rope_s", [128, 640], f32, kind="ExternalInput").ap()
    kT_d = nc.dram_tensor("kT", [B, 128, T], bf16, kind="ExternalInput").ap()
    v_d = nc.dram_tensor("vperm", [B, 128, T], bf16, kind="ExternalInput").ap()
    out_d = nc.dram_tensor("out", [BS, E], f16, kind="ExternalOutput").ap()

    with tile.TileContext(nc) as tc:
        from contextlib import ExitStack

        with ExitStack() as ctx:
            const = ctx.enter_context(tc.tile_pool(name="const", bufs=1))
            woutp = ctx.enter_context(tc.tile_pool(name="woutp", bufs=1))
            work = ctx.enter_context(tc.tile_pool(name="work", bufs=1))
            # kst: one rotation shared by the 4 w_in slices then the 8 kk
            # tiles -- w_in buffers are dead after the qkv projection, so
            # kk2..5 reuse them and every kk DMA can issue by ~25us (the
            # issue is then ring-paced, not PE-paced)
            kst = ctx.enter_context(tc.tile_pool(name="kst", bufs=10))
            epool = ctx.enter_context(tc.tile_pool(name="epool", bufs=2))
            opool = ctx.enter_context(tc.tile_pool(name="opool", bufs=3))
            # PSUM budget (8 banks): ps_sc 4x[128,512]=4 (scores rotate 4
            # quarter-banks so QK never waits on ACT; also transposes, bc,
            # warm-up, out-projection), ps_big 1x[128,1024]=2 (qkv proj),
            # ps_out 1 ([128,128]: two 64-col PV accumulators packed),
            # ps_den 1 ([1,512] denominator)
            ps_sc = ctx.enter_context(tc.tile_pool(name="ps_sc", bufs=4, space="PSUM"))
            ps_big = ctx.enter_context(
                tc.tile_pool(name="ps_big", bufs=1, space="PSUM")
            )
            ps_out = ctx.enter_context(
                tc.tile_pool(name="ps_out", bufs=1, space="PSUM")
            )
            ps_den = ctx.enter_context(
                tc.tile_pool(name="ps_den", bufs=1, space="PSUM")
            )

            # ---- constants
            ident = const.tile([128, 128], f32, tag="ident")
            masks.make_identity(nc, ident[:])
            ones_col = const.tile([128, 1], bf16, tag="ones_col")
            nc.vector.memset(ones_col[:], 1.0)
            ones_row = const.tile([1, 128], f32, tag="ones_row")
            nc.vector.memset(ones_row[:], 1.0)

            # ---- PE warm-up: ~3.5us of junk matmuls so HAM reaches K=8/8
            # before the first real matmul (w_in slice 0 lands ~6us in).
            wstat = const.tile([128, 128], bf16, tag="wstat")
            nc.vector.memset(wstat[:], 0.0)
            wmov = const.tile([128, 512], bf16, tag="wmov")
            nc.vector.memset(wmov[:], 0.0)
            warm_ps = ps_sc.tile([128, 512], f32, tag="sc", name="warm")
            for _ in range(40):
                nc.tensor.matmul(
                    warm_ps[:, 0:256], wstat[:], wmov[:, 0:256], start=True, stop=True
                )

            # ---- weight / activation loads. Both HWDGE queues (scalar +
            # sync) are kept loaded concurrently: one queue alone sustains
            # only ~330 GB/s, both together ~425 GB/s.
            xT = const.tile([128, KCH * 128], bf16, tag="xT")
            w_inT = [
                kst.tile([128, KPS * 768], bf16, tag="kst", name=f"w_inT{s}")
                for s in range(NSL)
            ]
            ropeC = const.tile([128, 640], f32, tag="ropeC")
            ropeS = const.tile([128, 640], f32, tag="ropeS")
            # single-queue schedule: ALL inputs ride the sync queue in exact
            # consumption order (one HWDGE ring sustains the full ~420 GB/s,
            # and ring FIFO = deterministic arrival order); the scalar engine
            # stream holds only activations so it can never stall on DMA
            nc.sync.dma_start(xT[:], xT_d[:])
            nc.sync.dma_start(ropeC[:], ropec_d[:])
            nc.sync.dma_start(ropeS[:], ropes_d[:])
            for s in range(NSL):
                nc.sync.dma_start(w_inT[s][:], win_d[s][:])
            # w_out column-slice tiles; DMAs issued late (inside batch loop)
            w_outT = [
                woutp.tile([128, R * 512], bf16, tag=f"w_outT{n}", name=f"w_outT{n}")
                for n in range(8)
            ]

            # ---- phase 1: qkv projection [128, 768] in psum
            qkv_ps = ps_big.tile([128, 1024], f32, tag="big", name="qkv")
            for s in range(NSL):
                for j in range(KPS):
                    k = s * KPS + j
                    nc.tensor.matmul(
                        qkv_ps[:, 0:512],
                        xT[:, k * 128 : (k + 1) * 128],
                        w_inT[s][:, j * 768 : j * 768 + 512],
                        start=(k == 0),
                        stop=(k == KCH - 1),
                    )
                    nc.tensor.matmul(
                        qkv_ps[:, 512:768],
                        xT[:, k * 128 : (k + 1) * 128],
                        w_inT[s][:, j * 768 + 512 : (j + 1) * 768],
                        start=(k == 0),
                        stop=(k == KCH - 1),
                    )

            # v block to sbuf (bf16), used to patch the v cache tail per batch
            # (patch is an SBUF->SBUF DMA: compute engines need quadrant-
            # aligned partition starts, DMA does not)
            v_sb = work.tile([128, 128], bf16, tag="v_sb")
            nc.vector.tensor_copy(v_sb[:], qkv_ps[:, 640:768])

            # ---- rope on q|k (cols 0:640): out = t*C + rot(t)*S
            rot = work.tile([128, 640], f32, tag="rot")
            rot4 = rot[:].rearrange("p (blk h j) -> p blk h j", blk=5, h=2)
            ps4 = qkv_ps[:, 0:640].rearrange("p (blk h j) -> p blk h j", blk=5, h=2)
            nc.vector.tensor_copy(rot4[:, :, 0, :], ps4[:, :, 1, :])
            nc.vector.tensor_copy(rot4[:, :, 1, :], ps4[:, :, 0, :])
            # stage rope tables via copies: TT ops can't carry a DMA-queue
            # semaphore wait, so the copies absorb the DMA dependency
            ropeCs = work.tile([128, 640], f32, tag="ropeCs")
            nc.vector.tensor_copy(ropeCs[:], ropeC[:])
            ropeSs = work.tile([128, 640], f32, tag="ropeSs")
            nc.vector.tensor_copy(ropeSs[:], ropeS[:])
            roped = work.tile([128, 640], f32, tag="roped")
            nc.vector.tensor_mul(roped[:], qkv_ps[:, 0:640], ropeCs[:])
            t2 = work.tile([128, 640], f32, tag="t2")
            nc.vector.tensor_mul(t2[:], rot[:], ropeSs[:])
            nc.vector.tensor_add(roped[:], roped[:], t2[:])

            # ---- transpose q heads -> q_allT [128(d), (b r s)] bf16
            q_allT = work.tile([128, B * R * S], bf16, tag="q_allT")
            qv = q_allT[:].rearrange("p (b r s) -> p b r s", b=B, r=R)
            for r in range(R):
                tp = ps_sc.tile([128, 512], f32, tag="sc")
                nc.tensor.transpose(
                    tp[:, 0:128], roped[:, r * 128 : (r + 1) * 128], ident[:]
                )
                nc.vector.tensor_copy(
                    qv[:, :, r, :], tp[:, 0:128].rearrange("p (b s) -> p b s", b=B)
                )
            # transpose new k -> kT_sb [128(d), (b s)] bf16
            kT_sb = work.tile([128, 128], bf16, tag="kT_sb")
            tpk = ps_sc.tile([128, 512], f32, tag="sc")
            nc.tensor.transpose(tpk[:, 0:128], roped[:, 512:640], ident[:])
            nc.vector.tensor_copy(kT_sb[:], tpk[:, 0:128])

            # attention outputs per head, [128(d), (b s)] bf16, all batches
            attn_allT = [
                work.tile([128, BS], bf16, tag=f"attn{r}", name=f"attn{r}")
                for r in range(R)
            ]

            # ---- phase 2: attention per batch
            pending = []  # (b, outT_ps, recip) awaiting bc+scale

            # single psum bank holds both in-flight PV accumulators (64 cols
            # each, alternating); start=True clears has_written bits bank-wide
            # but not data, and the neighbor group is always stopped by then
            outT_duo = ps_out.tile([128, 128], f32, tag="po", name="outT_duo")

            def flush_pending():
                while pending:
                    pb, p_recip = pending.pop(0)
                    lo = (pb % 2) * 64
                    bc_ps = ps_sc.tile([128, 512], f32, tag="sc", name="bc_ps")
                    nc.tensor.matmul(
                        bc_ps[:, 0:64], ones_row[:], p_recip[:], start=True, stop=True
                    )
                    bc_sb = epool.tile([128, 64], f32, tag="bc_sb", name="bc_sb")
                    nc.vector.tensor_copy(bc_sb[:], bc_ps[:, 0:64])
                    for r in range(R):
                        nc.vector.tensor_mul(
                            attn_allT[r][:, pb * S : (pb + 1) * S],
                            outT_duo[:, lo + r * S : lo + (r + 1) * S],
                            bc_sb[:, r * S : (r + 1) * S],
                        )

            # kv stream: kk/vv interleaved in consumption order on the sync
            # ring, reusing the w_in buffers (kst rotation). kk0/vv0 have no
            # buffer dependency, so gate them on w_in slice 0 (tiny copies)
            # to keep the scheduler from hoisting their issue ahead of the
            # w_in stream; all later tiles carry natural buffer-reuse deps.
            kk_tiles = {}
            vv_tiles = {}
            for pb in range(B):
                kk = kst.tile([128, T], bf16, tag="kst", name=f"kk{pb}")
                vv = kst.tile([128, T], bf16, tag="kst", name=f"vv{pb}")
                if pb == 0:
                    nc.vector.tensor_copy(kk[0:1, 0:2], w_inT[0][0:1, 0:2])
                    nc.vector.tensor_copy(vv[0:1, 0:2], w_inT[0][0:1, 0:2])
                nc.sync.dma_start(kk[:], kT_d[pb][:])
                nc.sync.dma_start(vv[:], v_d[pb][:])
                kk_tiles[pb] = kk
                vv_tiles[pb] = vv

            for b in range(B):
                kk = kk_tiles.pop(b)
                # overwrite stale tail keys with roped new keys (gpsimd DMA:
                # a vector copy here would stall the DVE stream on kv data)
                nc.gpsimd.dma_start(
                    kk[:, OFFSET:T], kT_sb[:, b * S : (b + 1) * S]
                )
                vv = vv_tiles.pop(b)
                # patch new v tokens: chunk 31, partitions 112..127
                nc.gpsimd.dma_start(
                    vv[112:128, 31 * 128 : 32 * 128], v_sb[b * S : (b + 1) * S, :]
                )

                expS = epool.tile([128, TCH * 64], bf16, tag="expS")
                denom_ps = ps_den.tile([1, 512], f32, tag="den")
                olo = (b % 2) * 64
                # QK into 4 rotating quarter-banks; exp chases per quarter
                for q in range(4):
                    sc = ps_sc.tile([128, 512], f32, tag="sc")
                    for tt in range(8):
                        t = q * 8 + tt
                        nc.tensor.matmul(
                            sc[:, tt * 64 : (tt + 1) * 64],
                            kk[:, t * 128 : (t + 1) * 128],
                            q_allT[:, b * 64 : (b + 1) * 64],
                            start=True,
                            stop=True,
                        )
                    nc.scalar.activation(
                        expS[:, q * 512 : (q + 1) * 512], sc[:], ActExp
                    )
                    if q == 1 and b == 4:
                        # w_out: gate each slice's DMA behind a tiny memset
                        # (WAW dep) so the scheduler cannot hoist them ahead
                        # of the kv stream on the sync ring
                        for n in range(8):
                            nc.vector.memset(w_outT[n][0:1, 0:1], 0.0)
                        for n in range(8):
                            nc.sync.dma_start(w_outT[n][:], wout_d[n][:])
                # PV accumulation over all 32 chunks
                for t in range(TCH):
                    nc.tensor.matmul(
                        outT_duo[:, olo : olo + 64],
                        vv[:, t * 128 : (t + 1) * 128],
                        expS[:, t * 64 : (t + 1) * 64],
                        start=(t == 0),
                        stop=(t == TCH - 1),
                    )
                # denominator: four wide N=512 matmuls, one accumulation
                # group -> [1, 512] partial sums
                for i in range(4):
                    nc.tensor.matmul(
                        denom_ps[:],
                        ones_col[:],
                        expS[:, i * 512 : (i + 1) * 512],
                        start=(i == 0),
                        stop=(i == 3),
                    )
                # normalize the previous batch (its reciprocal is long ready)
                flush_pending()
                # tree-reduce the 8 chunk-groups: [1,512] -> [1,64]
                # (TT ops can read at most one PSUM input, so stage to SBUF)
                dred = epool.tile([1, 512], f32, tag="dred", name="dred")
                nc.vector.tensor_copy(dred[:, 0:512], denom_ps[:])
                nc.vector.tensor_add(dred[:, 0:256], dred[:, 0:256], dred[:, 256:512])
                nc.vector.tensor_add(dred[:, 0:128], dred[:, 0:128], dred[:, 128:256])
                nc.vector.tensor_add(dred[:, 0:64], dred[:, 0:64], dred[:, 64:128])
                recip = epool.tile([1, 64], f32, tag="recip", name="recip")
                nc.vector.reciprocal(recip[:], dred[:, 0:64])
                pending.append((b, recip))
            flush_pending()

            # ---- phase 3: out projection, full batch [128, 4096]
            # out[bs, e_out] = sum_r attn_rT.T @ w_out[r-block, e_out-slice]
            ActCopy = mybir.ActivationFunctionType.Copy
            for n in range(8):
                po = ps_sc.tile([128, 512], f32, tag="sc")
                for r in range(R):
                    nc.tensor.matmul(
                        po[:],
                        attn_allT[r][:],
                        w_outT[n][:, r * 512 : (r + 1) * 512],
                        start=(r == 0),
                        stop=(r == R - 1),
                    )
                osb = opool.tile([128, 512], f16, tag="osb")
                # alternate the psum->sbuf drain between DVE and ACT so
                # consecutive slices overlap
                if n % 2 == 0:
                    nc.vector.tensor_copy(osb[:], po[:])
                else:
                    nc.scalar.activation(osb[:], po[:], ActCopy)
                nc.scalar.dma_start(out_d[:, n * 512 : (n + 1) * 512], osb[:])

    nc.compile()
    return nc


def _host_shards(x, w_in, w_out, k_cache, v_cache):
    """Per-core input dicts, pre-permuted for contiguous [128, N] DMAs."""
    x2 = np.ascontiguousarray(x.reshape(BS, E))
    xT_perm = (
        x2.T.reshape(KCH, 128, 128).transpose(1, 0, 2).reshape(128, KCH * 128)
    ).astype(BF16)

    # rope tables (identical on all cores); fold attn scale into q blocks
    inv_freq = 1.0 / (ROPE_BASE ** (np.arange(0, HD, 2, dtype=np.float64) / HD))
    pos = (OFFSET + np.arange(S)).astype(np.float64)
    ang = pos[:, None] * inv_freq[None, :]          # [S, 64]
    cos16 = np.cos(ang).astype(np.float32)
    sin16 = np.sin(ang).astype(np.float32)
    scale = np.float32(1.0 / np.sqrt(HD))
    C = np.zeros((128, 640), np.float32)
    Sn = np.zeros((128, 640), np.float32)
    srow = np.arange(128) % S                        # partition p=(b,s) -> s
    for blk in range(5):
        blk_scale = scale if blk < 4 else np.float32(1.0)
        C[:, blk * 128 : blk * 128 + 64] = cos16[srow] * blk_scale
        C[:, blk * 128 + 64 : blk * 128 + 128] = cos16[srow] * blk_scale
        Sn[:, blk * 128 : blk * 128 + 64] = -sin16[srow] * blk_scale
        Sn[:, blk * 128 + 64 : blk * 128 + 128] = sin16[srow] * blk_scale

    shards = []
    for g in range(NCORES):
        rows = np.concatenate(
            [
                w_in[QF * g : QF * (g + 1)],
                w_in[E + HD * g : E + HD * (g + 1)],
                w_in[E + HKV * HD + HD * g : E + HKV * HD + HD * (g + 1)],
            ],
            axis=0,
        )  # [768, 4096]
        w_inT_perm = (
            rows.T.reshape(KCH, 128, 768)
            .transpose(1, 0, 2)
            .reshape(128, KCH * 768)
        ).astype(BF16)
        w_inT_sl = np.ascontiguousarray(
            w_inT_perm.reshape(128, NSL, KPS * 768).transpose(1, 0, 2)
        )  # [NSL, 128, KPS*768]
        w_outT = np.ascontiguousarray(
            w_out[:, QF * g : QF * (g + 1)].T
        )  # [512 e_in, 4096 e_out]
        wout8 = np.empty((8, 128, R * 512), np.float32)
        for n in range(8):
            for r in range(R):
                wout8[n, :, r * 512 : (r + 1) * 512] = w_outT[
                    r * 128 : (r + 1) * 128, n * 512 : (n + 1) * 512
                ]
        kT = np.ascontiguousarray(
            k_cache[:, :, g, :].transpose(0, 2, 1)
        ).astype(BF16)  # [B, 128(d), T]
        vperm = np.ascontiguousarray(
            v_cache[:, :, g, :]
            .reshape(B, TCH, 128, HD)
            .transpose(0, 2, 1, 3)
            .reshape(B, 128, T)
        ).astype(BF16)  # [B, 128(t_in), (chunk d)]
        shards.append(
            {
                "xT": xT_perm,
                "w_inT": w_inT_sl,
                "w_outT": wout8.astype(BF16),
                "rope_c": C,
                "rope_s": Sn,
                "kT": kT,
                "vperm": vperm,
            }
        )
    return shards


def _get_nc():
    if "nc" not in _CACHED:
        _CACHED["nc"] = _build_program()
    return _CACHED["nc"]


def run_on_hw(in_maps, trace=False, **kw):
    from concourse import bass_utils

    nc = _get_nc()
    return bass_utils.run_bass_kernel_spmd(
        nc, in_maps, core_ids=list(range(NCORES)), trace=trace, **kw
    )


def kernel(x, w_in, w_out, k_cache, v_cache, offset):
    assert int(offset) == OFFSET and x.shape == (B, S, E)
    shards = _host_shards(
        np.asarray(x, np.float32),
        np.asarray(w_in, np.float32),
        np.asarray(w_out, np.float32),
        np.asarray(k_cache, np.float32),
        np.asarray(v_cache, np.float32),
    )
    res = run_on_hw(shards)
    out = np.zeros((BS, E), np.float64)
    for g in range(NCORES):
        out += np.asarray(res.results[g]["out"], np.float64)
    return out.astype(np.float32).reshape(B, S, E)


# revision 37
# speedup vs baseline: 1.1404x; 1.0549x over previous
"""GQA decode attention kernel for Trainium2, tensor-parallel over 8 kv heads.

Contract: kernel(**inputs) takes FULL inputs (numpy), returns FULL output.
Shapes are hardcoded: x[8,16,4096], w_in[6144,4096], w_out[4096,4096],
k_cache[8,4096,8,128], v_cache[8,4096,8,128], offset=4080.

Per-core (core g owns kv head g, q heads 4g..4g+3):
  qkv = x @ w_in_g.T            -> [128, 768] (q 512 | k 128 | v 128)
  rope(q, k), scatter new k/v into cache tail (T=4096)
  scoresT[t, (r,s)] = kkT chunks.T @ qT    (PE, per batch)
  expS = exp(scores)            (ACT, no max-sub: |scores| < ~8)
  denom = ones.T @ expS         (4 wide accumulating MMs + DVE tree)
  outT = vv.T @ expS            (PE accumulate) ; scaled by 1/denom
  partial = attn.T stationary vs w_out column slices -> [128, 4096]
Host sums the 8 partials.

v2 schedule: PE warm-up burst, w_in split in 4 slice tiles (fine-grained
DMA deps), kv stream gated behind w_in via dummy DMA dep, w_out streamed
late as 8 column-slice tiles, full-batch out projection.
"""

import os
import sys

for _p in ("/opt/trn_rl_repo", "/root/.axon_site/_ro/trn_rl_repo"):
    if os.path.isdir(_p) and _p not in sys.path:
        sys.path.insert(0, _p)

import numpy as np
import ml_dtypes

BF16 = ml_dtypes.bfloat16

B, S, E = 8, 16, 4096
HQ, HKV, HD = 32, 8, 128
R = HQ // HKV          # 4 q heads per kv head
T = 4096               # cache length == offset + S
OFFSET = 4080
NCORES = 8
ROPE_BASE = 10000.0
BS = B * S             # 128 rows
QF = R * HD            # 512 q features per core
KCH = E // 128         # 32 contraction chunks for qkv proj
TCH = T // 128         # 32 T chunks
NSL = 8                # w_in DMA slices
KPS = KCH // NSL       # k-chunks per slice

_CACHED = {}


def _build_program():
    """Build the Bass program once (same program for all cores)."""
    from concourse import bacc, bass, masks, mybir
    from concourse import tile

    f32 = mybir.dt.float32
    bf16 = mybir.dt.bfloat16
    f16 = mybir.dt.float16
    ActExp = mybir.ActivationFunctionType.Exp

    nc = bacc.Bacc(
        "TRN2",
        target_bir_lowering=False,
        debug=False,
        enable_asserts=False,
        num_devices=NCORES,
    )

    # DRAM I/O (per-core shards, host pre-permuted so every DMA is a plain
    # [128, N] contiguous-per-partition transfer)
    xT_d = nc.dram_tensor("xT", [128, KCH * 128], bf16, kind="ExternalInput").ap()
    win_d = nc.dram_tensor(
        "w_inT", [NSL, 128, KPS * 768], bf16, kind="ExternalInput"
    ).ap()
    wout_d = nc.dram_tensor(
        "w_outT", [8, 128, R * 512], bf16, kind="ExternalInput"
    ).ap()
    ropec_d = nc.dram_tensor("rope_c", [128, 640], f32, kind="ExternalInput").ap()
    ropes_d = nc.dram_tensor("rope_s", [128, 640], f32, kind="ExternalInput").ap()
    kT_d = nc.dram_tensor("kT", [B, 128, T], bf16, kind="ExternalInput").ap()
    v_d = nc.dram_tensor("vperm", [B, 128, T], bf16, kind="ExternalInput").ap()
    out_d = nc.dram_tensor("out", [BS, E], f16, kind="ExternalOutput").ap()

    with tile.TileContext(nc) as tc:
        from contextlib import ExitStack

        with ExitStack() as ctx:
            const = ctx.enter_context(tc.tile_pool(name="const", bufs=1))
            woutp = ctx.enter_context(tc.tile_pool(name="woutp", bufs=1))
            work = ctx.enter_context(tc.tile_pool(name="work", bufs=1))
            # kst: one rotation shared by the 4 w_in slices then the 8 kk
            # tiles -- w_in buffers are dead after the qkv projection, so
            # kk2..5 reuse them and every kk DMA can issue by ~25us (the
            # issue is then ring-paced, not PE-paced)
            kst = ctx.enter_context(tc.tile_pool(name="kst", bufs=10))
            epool = ctx.enter_context(tc.tile_pool(name="epool", bufs=2))
            opool = ctx.enter_context(tc.tile_pool(name="opool", bufs=3))
            # PSUM budget (8 banks): ps_sc 4x[128,512]=4 (scores rotate 4
            # quarter-banks so QK never waits on ACT; also transposes, bc,
            # warm-up, out-projection), ps_big 1x[128,1024]=2 (qkv proj),
            # ps_out 1 ([128,128]: two 64-col PV accumulators packed),
            # ps_den 1 ([1,512] denominator)
            ps_sc = ctx.enter_context(tc.tile_pool(name="ps_sc", bufs=2, space="PSUM"))
            ps_big = ctx.enter_context(
                tc.tile_pool(name="ps_big", bufs=1, space="PSUM")
            )
            ps_out = ctx.enter_context(
                tc.tile_pool(name="ps_out", bufs=1, space="PSUM")
            )
            ps_den = ctx.enter_context(
                tc.tile_pool(name="ps_den", bufs=1, space="PSUM")
            )

            # ---- constants
            ident = const.tile([128, 128], f32, tag="ident")
            masks.make_identity(nc, ident[:])
            ones_col = const.tile([128, 1], bf16, tag="ones_col")
            nc.vector.memset(ones_col[:], 1.0)
            ones_row = const.tile([1, 128], bf16, tag="ones_row")
            nc.vector.memset(ones_row[:], 1.0)

            # ---- PE warm-up: ~3.5us of junk matmuls so HAM reaches K=8/8
            # before the first real matmul (w_in slice 0 lands ~6us in).
            wstat = const.tile([128, 128], bf16, tag="wstat")
            nc.vector.memset(wstat[:], 0.0)
            wmov = const.tile([128, 512], bf16, tag="wmov")
            nc.vector.memset(wmov[:], 0.0)
            warm_ps = ps_sc.tile([128, 512], f32, tag="sc", name="warm")
            for _ in range(56):
                nc.tensor.matmul(
                    warm_ps[:, 0:256], wstat[:], wmov[:, 0:256], start=True, stop=True
                )

            # ---- weight / activation loads. Both HWDGE queues (scalar +
            # sync) are kept loaded concurrently: one queue alone sustains
            # only ~330 GB/s, both together ~425 GB/s.
            xT = const.tile([128, KCH * 128], bf16, tag="xT")
            w_inT = [
                kst.tile([128, KPS * 768], bf16, tag="kst", name=f"w_inT{s}")
                for s in range(NSL)
            ]
            ropeC = const.tile([128, 640], f32, tag="ropeC")
            ropeS = const.tile([128, 640], f32, tag="ropeS")
            # single-queue schedule: ALL inputs ride the sync queue in exact
            # consumption order (one HWDGE ring sustains the full ~420 GB/s,
            # and ring FIFO = deterministic arrival order); the scalar engine
            # stream holds only activations so it can never stall on DMA
            nc.sync.dma_start(xT[:], xT_d[:])
            nc.sync.dma_start(ropeC[:], ropec_d[:])
            nc.sync.dma_start(ropeS[:], ropes_d[:])
            for s in range(NSL):
                nc.sync.dma_start(w_inT[s][:], win_d[s][:])
            # w_out column-slice tiles; DMAs issued late (inside batch loop)
            w_outT = [
                woutp.tile([128, R * 512], bf16, tag=f"w_outT{n}", name=f"w_outT{n}")
                for n in range(8)
            ]

            # ---- phase 1: qkv projection [128, 768] in psum
            qkv_ps = ps_big.tile([128, 1024], f32, tag="big", name="qkv")
            for s in range(NSL):
                for j in range(KPS):
                    k = s * KPS + j
                    nc.tensor.matmul(
                        qkv_ps[:, 0:512],
                        xT[:, k * 128 : (k + 1) * 128],
                        w_inT[s][:, j * 768 : j * 768 + 512],
                        start=(k == 0),
                        stop=(k == KCH - 1),
                    )
                    nc.tensor.matmul(
                        qkv_ps[:, 512:768],
                        xT[:, k * 128 : (k + 1) * 128],
                        w_inT[s][:, j * 768 + 512 : (j + 1) * 768],
                        start=(k == 0),
                        stop=(k == KCH - 1),
                    )

            # v block to sbuf (bf16), used to patch the v cache tail per batch
            # (patch is an SBUF->SBUF DMA: compute engines need quadrant-
            # aligned partition starts, DMA does not)
            v_sb = work.tile([128, 128], bf16, tag="v_sb")
            nc.vector.tensor_copy(v_sb[:], qkv_ps[:, 640:768])

            # ---- rope on q|k (cols 0:640): out = t*C + rot(t)*S
            rot = work.tile([128, 640], f32, tag="rot")
            rot4 = rot[:].rearrange("p (blk h j) -> p blk h j", blk=5, h=2)
            ps4 = qkv_ps[:, 0:640].rearrange("p (blk h j) -> p blk h j", blk=5, h=2)
            nc.vector.tensor_copy(rot4[:, :, 0, :], ps4[:, :, 1, :])
            nc.vector.tensor_copy(rot4[:, :, 1, :], ps4[:, :, 0, :])
            # stage rope tables via copies: TT ops can't carry a DMA-queue
            # semaphore wait, so the copies absorb the DMA dependency
            ropeCs = work.tile([128, 640], f32, tag="ropeCs")
            nc.vector.tensor_copy(ropeCs[:], ropeC[:])
            ropeSs = work.tile([128, 640], f32, tag="ropeSs")
            nc.vector.tensor_copy(ropeSs[:], ropeS[:])
            roped = work.tile([128, 640], f32, tag="roped")
            nc.vector.tensor_mul(roped[:], qkv_ps[:, 0:640], ropeCs[:])
            t2 = work.tile([128, 640], f32, tag="t2")
            nc.vector.tensor_mul(t2[:], rot[:], ropeSs[:])
            nc.vector.tensor_add(roped[:], roped[:], t2[:])

            # ---- transpose q heads -> q_allT [128(d), (b r s)] bf16
            q_allT = work.tile([128, B * R * S], bf16, tag="q_allT")
            qv = q_allT[:].rearrange("p (b r s) -> p b r s", b=B, r=R)
            for r in range(R):
                tp = ps_sc.tile([128, 512], f32, tag="sc")
                nc.tensor.transpose(
                    tp[:, 0:128], roped[:, r * 128 : (r + 1) * 128], ident[:]
                )
                nc.vector.tensor_copy(
                    qv[:, :, r, :], tp[:, 0:128].rearrange("p (b s) -> p b s", b=B)
                )
            # transpose new k -> kT_sb [128(d), (b s)] bf16
            kT_sb = work.tile([128, 128], bf16, tag="kT_sb")
            tpk = ps_sc.tile([128, 512], f32, tag="sc")
            nc.tensor.transpose(tpk[:, 0:128], roped[:, 512:640], ident[:])
            nc.vector.tensor_copy(kT_sb[:], tpk[:, 0:128])

            # attention outputs per head, [128(d), (b s)] bf16, all batches
            attn_allT = [
                work.tile([128, BS], bf16, tag=f"attn{r}", name=f"attn{r}")
                for r in range(R)
            ]

            # ---- phase 2: attention per batch
            pending = []  # (b, outT_ps, recip) awaiting bc+scale

            # single psum bank holds both in-flight PV accumulators (64 cols
            # each, alternating); start=True clears has_written bits bank-wide
            # but not data, and the neighbor group is always stopped by then
            outT_duo = ps_out.tile([128, 128], f32, tag="po", name="outT_duo")

            def flush_pending():
                while pending:
                    pb, p_recip = pending.pop(0)
                    lo = (pb % 2) * 64
                    bc_ps = ps_sc.tile([128, 512], f32, tag="sc", name="bc_ps")
                    nc.tensor.matmul(
                        bc_ps[:, 0:64], ones_row[:], p_recip[:], start=True, stop=True
                    )
                    bc_sb = epool.tile([128, 64], f32, tag="bc_sb", name="bc_sb")
                    nc.vector.tensor_copy(bc_sb[:], bc_ps[:, 0:64])
                    for r in range(R):
                        nc.vector.tensor_mul(
                            attn_allT[r][:, pb * S : (pb + 1) * S],
                            outT_duo[:, lo + r * S : lo + (r + 1) * S],
                            bc_sb[:, r * S : (r + 1) * S],
                        )

            # kv stream: kk/vv interleaved in consumption order on the sync
            # ring, reusing the w_in buffers (kst rotation). kk0/vv0 have no
            # buffer dependency, so gate them on w_in slice 0 (tiny copies)
            # to keep the scheduler from hoisting their issue ahead of the
            # w_in stream; all later tiles carry natural buffer-reuse deps.
            kk_tiles = {}
            vv_tiles = {}
            for pb in range(B):
                kk = kst.tile([128, T], bf16, tag="kst", name=f"kk{pb}")
                vv = kst.tile([128, T], bf16, tag="kst", name=f"vv{pb}")
                if pb == 0:
                    nc.vector.tensor_copy(kk[0:1, 0:2], w_inT[0][0:1, 0:2])
                    nc.vector.tensor_copy(vv[0:1, 0:2], w_inT[0][0:1, 0:2])
                nc.sync.dma_start(kk[:], kT_d[pb][:])
                nc.sync.dma_start(vv[:], v_d[pb][:])
                kk_tiles[pb] = kk
                vv_tiles[pb] = vv

            for b in range(B):
                kk = kk_tiles.pop(b)
                # overwrite stale tail keys with roped new keys (gpsimd DMA:
                # a vector copy here would stall the DVE stream on kv data)
                nc.gpsimd.dma_start(
                    kk[:, OFFSET:T], kT_sb[:, b * S : (b + 1) * S]
                )
                vv = vv_tiles.pop(b)
                # patch new v tokens: chunk 31, partitions 112..127
                nc.gpsimd.dma_start(
                    vv[112:128, 31 * 128 : 32 * 128], v_sb[b * S : (b + 1) * S, :]
                )

                expS = epool.tile([128, TCH * 64], bf16, tag="expS")
                denom_ps = ps_den.tile([1, 512], f32, tag="den")
                olo = (b % 2) * 64
                # QK into 2 rotating half-banks (bufs=2: QK(b,h) reuses the
                # buffer exp(b-1,h) read, so no intra-batch ACT serialization)
                # with ONE wide [128,1024] exp per half
                for h in range(2):
                    sc = ps_sc.tile([128, 1024], f32, tag="sc")
                    for tt in range(16):
                        t = h * 16 + tt
                        nc.tensor.matmul(
                            sc[:, tt * 64 : (tt + 1) * 64],
                            kk[:, t * 128 : (t + 1) * 128],
                            q_allT[:, b * 64 : (b + 1) * 64],
                            start=True,
                            stop=True,
                        )
                    nc.scalar.activation(
                        expS[:, h * 1024 : (h + 1) * 1024], sc[:], ActExp
                    )
                    if h == 1 and b == 4:
                        # w_out: gate each slice's DMA behind a tiny memset
                        # (WAW dep) so the scheduler cannot hoist them ahead
                        # of the kv stream on the sync ring
                        for n in range(8):
                            nc.vector.memset(w_outT[n][0:1, 0:1], 0.0)
                        for n in range(8):
                            nc.sync.dma_start(w_outT[n][:], wout_d[n][:])
                # PV accumulation over all 32 chunks
                for t in range(TCH):
                    nc.tensor.matmul(
                        outT_duo[:, olo : olo + 64],
                        vv[:, t * 128 : (t + 1) * 128],
                        expS[:, t * 64 : (t + 1) * 64],
                        start=(t == 0),
                        stop=(t == TCH - 1),
                    )
                # denominator: four wide N=512 matmuls, one accumulation
                # group -> [1, 512] partial sums
                for i in range(4):
                    nc.tensor.matmul(
                        denom_ps[:],
                        ones_col[:],
                        expS[:, i * 512 : (i + 1) * 512],
                        start=(i == 0),
                        stop=(i == 3),
                    )
                # normalize the previous batch (its reciprocal is long ready)
                flush_pending()
                # tree-reduce the 8 chunk-groups: [1,512] -> [1,64]
                # (TT ops can read at most one PSUM input, so stage to SBUF)
                dred = epool.tile([1, 512], f32, tag="dred", name="dred")
                nc.vector.tensor_copy(dred[:, 0:512], denom_ps[:])
                nc.vector.tensor_add(dred[:, 0:256], dred[:, 0:256], dred[:, 256:512])
                nc.vector.tensor_add(dred[:, 0:128], dred[:, 0:128], dred[:, 128:256])
                nc.vector.tensor_add(dred[:, 0:64], dred[:, 0:64], dred[:, 64:128])
                recip = epool.tile([1, 64], bf16, tag="recip", name="recip")
                with nc.allow_low_precision(
                    reason="1/denom in bf16: 0.4% scale noise, within tolerance"
                ):
                    nc.vector.reciprocal(recip[:], dred[:, 0:64])
                pending.append((b, recip))
            flush_pending()

            # ---- phase 3: out projection, full batch [128, 4096]
            # out[bs, e_out] = sum_r attn_rT.T @ w_out[r-block, e_out-slice]
            ActCopy = mybir.ActivationFunctionType.Copy
            for n in range(8):
                po = ps_sc.tile([128, 512], f32, tag="sc")
                for r in range(R):
                    nc.tensor.matmul(
                        po[:],
                        attn_allT[r][:],
                        w_outT[n][:, r * 512 : (r + 1) * 512],
                        start=(r == 0),
                        stop=(r == R - 1),
                    )
                osb = opool.tile([128, 512], f16, tag="osb")
                # alternate the psum->sbuf drain between DVE and ACT so
                # consecutive slices overlap
                if n % 2 == 0:
                    nc.vector.tensor_copy(osb[:], po[:])
                else:
                    nc.scalar.activation(osb[:], po[:], ActCopy)
                nc.scalar.dma_start(out_d[:, n * 512 : (n + 1) * 512], osb[:])

    nc.compile()
    return nc


def _host_shards(x, w_in, w_out, k_cache, v_cache):
    """Per-core input dicts, pre-permuted for contiguous [128, N] DMAs."""
    x2 = np.ascontiguousarray(x.reshape(BS, E))
    xT_perm = (
        x2.T.reshape(KCH, 128, 128).transpose(1, 0, 2).reshape(128, KCH * 128)
    ).astype(BF16)

    # rope tables (identical on all cores); fold attn scale into q blocks
    inv_freq = 1.0 / (ROPE_BASE ** (np.arange(0, HD, 2, dtype=np.float64) / HD))
    pos = (OFFSET + np.arange(S)).astype(np.float64)
    ang = pos[:, None] * inv_freq[None, :]          # [S, 64]
    cos16 = np.cos(ang).astype(np.float32)
    sin16 = np.sin(ang).astype(np.float32)
    scale = np.float32(1.0 / np.sqrt(HD))
    C = np.zeros((128, 640), np.float32)
    Sn = np.zeros((128, 640), np.float32)
    srow = np.arange(128) % S                        # partition p=(b,s) -> s
    for blk in range(5):
        blk_scale = scale if blk < 4 else np.float32(1.0)
        C[:, blk * 128 : blk * 128 + 64] = cos16[srow] * blk_scale
        C[:, blk * 128 + 64 : blk * 128 + 128] = cos16[srow] * blk_scale
        Sn[:, blk * 128 : blk * 128 + 64] = -sin16[srow] * blk_scale
        Sn[:, blk * 128 + 64 : blk * 128 + 128] = sin16[srow] * blk_scale

    shards = []
    for g in range(NCORES):
        rows = np.concatenate(
            [
                w_in[QF * g : QF * (g + 1)],
                w_in[E + HD * g : E + HD * (g + 1)],
                w_in[E + HKV * HD + HD * g : E + HKV * HD + HD * (g + 1)],
            ],
            axis=0,
        )  # [768, 4096]
        w_inT_perm = (
            rows.T.reshape(KCH, 128, 768)
            .transpose(1, 0, 2)
            .reshape(128, KCH * 768)
        ).astype(BF16)
        w_inT_sl = np.ascontiguousarray(
            w_inT_perm.reshape(128, NSL, KPS * 768).transpose(1, 0, 2)
        )  # [NSL, 128, KPS*768]
        w_outT = np.ascontiguousarray(
            w_out[:, QF * g : QF * (g + 1)].T
        )  # [512 e_in, 4096 e_out]
        wout8 = np.empty((8, 128, R * 512), np.float32)
        for n in range(8):
            for r in range(R):
                wout8[n, :, r * 512 : (r + 1) * 512] = w_outT[
                    r * 128 : (r + 1) * 128, n * 512 : (n + 1) * 512
                ]
        kT = np.ascontiguousarray(
            k_cache[:, :, g, :].transpose(0, 2, 1)
        ).astype(BF16)  # [B, 128(d), T]
        vperm = np.ascontiguousarray(
            v_cache[:, :, g, :]
            .reshape(B, TCH, 128, HD)
            .transpose(0, 2, 1, 3)
            .reshape(B, 128, T)
        ).astype(BF16)  # [B, 128(t_in), (chunk d)]
        shards.append(
            {
                "xT": xT_perm,
                "w_inT": w_inT_sl,
                "w_outT": wout8.astype(BF16),
                "rope_c": C,
                "rope_s": Sn,
                "kT": kT,
                "vperm": vperm,
            }
        )
    return shards


def _get_nc():
    if "nc" not in _CACHED:
        _CACHED["nc"] = _build_program()
    return _CACHED["nc"]


def run_on_hw(in_maps, trace=False, **kw):
    from concourse import bass_utils

    nc = _get_nc()
    return bass_utils.run_bass_kernel_spmd(
        nc, in_maps, core_ids=list(range(NCORES)), trace=trace, **kw
    )


def kernel(x, w_in, w_out, k_cache, v_cache, offset):
    assert int(offset) == OFFSET and x.shape == (B, S, E)
    shards = _host_shards(
        np.asarray(x, np.float32),
        np.asarray(w_in, np.float32),
        np.asarray(w_out, np.float32),
        np.asarray(k_cache, np.float32),
        np.asarray(v_cache, np.float32),
    )
    res = run_on_hw(shards)
    out = np.zeros((BS, E), np.float64)
    for g in range(NCORES):
        out += np.asarray(res.results[g]["out"], np.float64)
    return out.astype(np.float32).reshape(B, S, E)


# revision 39
# speedup vs baseline: 1.2297x; 1.0783x over previous
"""GQA decode attention kernel for Trainium2, tensor-parallel over 8 kv heads.

Contract: kernel(**inputs) takes FULL inputs (numpy), returns FULL output.
Shapes are hardcoded: x[8,16,4096], w_in[6144,4096], w_out[4096,4096],
k_cache[8,4096,8,128], v_cache[8,4096,8,128], offset=4080.

Per-core (core g owns kv head g, q heads 4g..4g+3):
  qkv = x @ w_in_g.T            -> [128, 768] (q 512 | k 128 | v 128)
  rope(q, k), scatter new k/v into cache tail (T=4096)
  scoresT[t, (r,s)] = kkT chunks.T @ qT    (PE, per batch)
  expS = exp(scores)            (ACT, no max-sub: |scores| < ~8)
  denom = ones.T @ expS         (4 wide accumulating MMs + DVE tree)
  outT = vv.T @ expS            (PE accumulate) ; scaled by 1/denom
  partial = attn.T stationary vs w_out column slices -> [128, 4096]
Host sums the 8 partials.

v2 schedule: PE warm-up burst, w_in split in 4 slice tiles (fine-grained
DMA deps), kv stream gated behind w_in via dummy DMA dep, w_out streamed
late as 8 column-slice tiles, full-batch out projection.
"""

import os
import sys

for _p in ("/opt/trn_rl_repo", "/root/.axon_site/_ro/trn_rl_repo"):
    if os.path.isdir(_p) and _p not in sys.path:
        sys.path.insert(0, _p)

import numpy as np
import ml_dtypes

BF16 = ml_dtypes.bfloat16

B, S, E = 8, 16, 4096
HQ, HKV, HD = 32, 8, 128
R = HQ // HKV          # 4 q heads per kv head
T = 4096               # cache length == offset + S
OFFSET = 4080
NCORES = 8
ROPE_BASE = 10000.0
BS = B * S             # 128 rows
QF = R * HD            # 512 q features per core
KCH = E // 128         # 32 contraction chunks for qkv proj
TCH = T // 128         # 32 T chunks
NSL = 8                # w_in DMA slices
KPS = KCH // NSL       # k-chunks per slice

_CACHED = {}


def _build_program():
    """Build the Bass program once (same program for all cores)."""
    from concourse import bacc, bass, masks, mybir
    from concourse import tile

    f32 = mybir.dt.float32
    bf16 = mybir.dt.bfloat16
    f16 = mybir.dt.float16
    ActExp = mybir.ActivationFunctionType.Exp

    nc = bacc.Bacc(
        "TRN2",
        target_bir_lowering=False,
        debug=False,
        enable_asserts=False,
        num_devices=NCORES,
    )

    # DRAM I/O (per-core shards, host pre-permuted so every DMA is a plain
    # [128, N] contiguous-per-partition transfer)
    xT_d = nc.dram_tensor("xT", [128, KCH * 128], bf16, kind="ExternalInput").ap()
    win_d = nc.dram_tensor(
        "w_inT", [NSL, 128, KPS * 768], bf16, kind="ExternalInput"
    ).ap()
    wout_d = nc.dram_tensor(
        "w_outT", [8, 128, R * 512], bf16, kind="ExternalInput"
    ).ap()
    ropec_d = nc.dram_tensor("rope_c", [128, 640], f32, kind="ExternalInput").ap()
    ropes_d = nc.dram_tensor("rope_s", [128, 640], f32, kind="ExternalInput").ap()
    kT_d = nc.dram_tensor("kT", [B, 128, T], bf16, kind="ExternalInput").ap()
    v_d = nc.dram_tensor("vperm", [B, 128, T], bf16, kind="ExternalInput").ap()
    out_d = nc.dram_tensor("out", [BS, E], f16, kind="ExternalOutput").ap()

    with tile.TileContext(nc) as tc:
        from contextlib import ExitStack

        with ExitStack() as ctx:
            const = ctx.enter_context(tc.tile_pool(name="const", bufs=1))
            woutp = ctx.enter_context(tc.tile_pool(name="woutp", bufs=1))
            work = ctx.enter_context(tc.tile_pool(name="work", bufs=1))
            # kst: one rotation shared by the 4 w_in slices then the 8 kk
            # tiles -- w_in buffers are dead after the qkv projection, so
            # kk2..5 reuse them and every kk DMA can issue by ~25us (the
            # issue is then ring-paced, not PE-paced)
            kst = ctx.enter_context(tc.tile_pool(name="kst", bufs=10))
            epool = ctx.enter_context(tc.tile_pool(name="epool", bufs=2))
            opool = ctx.enter_context(tc.tile_pool(name="opool", bufs=3))
            # PSUM budget (8 banks): ps_sc 4x[128,512]=4 (scores rotate 4
            # quarter-banks so QK never waits on ACT; also transposes, bc,
            # warm-up, out-projection), ps_big 1x[128,1024]=2 (qkv proj),
            # ps_out 1 ([128,128]: two 64-col PV accumulators packed),
            # ps_den 1 ([1,512] denominator)
            ps_sc = ctx.enter_context(tc.tile_pool(name="ps_sc", bufs=2, space="PSUM"))
            ps_big = ctx.enter_context(
                tc.tile_pool(name="ps_big", bufs=1, space="PSUM")
            )
            ps_out = ctx.enter_context(
                tc.tile_pool(name="ps_out", bufs=1, space="PSUM")
            )
            ps_den = ctx.enter_context(
                tc.tile_pool(name="ps_den", bufs=1, space="PSUM")
            )

            # ---- constants
            ident = const.tile([128, 128], f32, tag="ident")
            masks.make_identity(nc, ident[:])
            ident_bf = const.tile([128, 128], bf16, tag="ident_bf")
            nc.vector.tensor_copy(ident_bf[:], ident[:])
            ones_col = const.tile([128, 1], bf16, tag="ones_col")
            nc.vector.memset(ones_col[:], 1.0)
            ones_row = const.tile([1, 128], bf16, tag="ones_row")
            nc.vector.memset(ones_row[:], 1.0)

            # ---- PE warm-up: ~3.5us of junk matmuls so HAM reaches K=8/8
            # before the first real matmul (w_in slice 0 lands ~6us in).
            wstat = const.tile([128, 128], bf16, tag="wstat")
            nc.vector.memset(wstat[:], 0.0)
            wmov = const.tile([128, 512], bf16, tag="wmov")
            nc.vector.memset(wmov[:], 0.0)
            warm_ps = ps_sc.tile([128, 512], f32, tag="sc", name="warm")
            for _ in range(56):
                nc.tensor.matmul(
                    warm_ps[:, 0:256], wstat[:], wmov[:, 0:256], start=True, stop=True
                )

            # ---- weight / activation loads. Both HWDGE queues (scalar +
            # sync) are kept loaded concurrently: one queue alone sustains
            # only ~330 GB/s, both together ~425 GB/s.
            xT = const.tile([128, KCH * 128], bf16, tag="xT")
            w_inT = [
                kst.tile([128, KPS * 768], bf16, tag="kst", name=f"w_inT{s}")
                for s in range(NSL)
            ]
            ropeC = const.tile([128, 640], f32, tag="ropeC")
            ropeS = const.tile([128, 640], f32, tag="ropeS")
            # single-queue schedule: ALL inputs ride the sync queue in exact
            # consumption order (one HWDGE ring sustains the full ~420 GB/s,
            # and ring FIFO = deterministic arrival order); the scalar engine
            # stream holds only activations so it can never stall on DMA
            nc.sync.dma_start(xT[:], xT_d[:])
            nc.sync.dma_start(ropeC[:], ropec_d[:])
            nc.sync.dma_start(ropeS[:], ropes_d[:])
            for s in range(NSL):
                nc.sync.dma_start(w_inT[s][:], win_d[s][:])
            # w_out column-slice tiles; DMAs issued late (inside batch loop)
            w_outT = [
                woutp.tile([128, R * 512], bf16, tag=f"w_outT{n}", name=f"w_outT{n}")
                for n in range(8)
            ]

            # ---- phase 1: qkv projection [128, 768] in psum
            qkv_ps = ps_big.tile([128, 1024], f32, tag="big", name="qkv")
            for s in range(NSL):
                for j in range(KPS):
                    k = s * KPS + j
                    nc.tensor.matmul(
                        qkv_ps[:, 0:512],
                        xT[:, k * 128 : (k + 1) * 128],
                        w_inT[s][:, j * 768 : j * 768 + 512],
                        start=(k == 0),
                        stop=(k == KCH - 1),
                    )
                    nc.tensor.matmul(
                        qkv_ps[:, 512:768],
                        xT[:, k * 128 : (k + 1) * 128],
                        w_inT[s][:, j * 768 + 512 : (j + 1) * 768],
                        start=(k == 0),
                        stop=(k == KCH - 1),
                    )

            # stage rope tables via copies (early: only the rope DMAs gate
            # them): TT ops can't carry a DMA-queue semaphore wait, so the
            # copies absorb the DMA dependency
            ropeCs = work.tile([128, 640], f32, tag="ropeCs")
            nc.vector.tensor_copy(ropeCs[:], ropeC[:])
            ropeSs = work.tile([128, 640], f32, tag="ropeSs")
            nc.vector.tensor_copy(ropeSs[:], ropeS[:])

            # v block to sbuf (bf16), used to patch the v cache tail per batch
            # (patch is an SBUF->SBUF DMA: compute engines need quadrant-
            # aligned partition starts, DMA does not)
            v_sb = work.tile([128, 128], bf16, tag="v_sb")
            nc.vector.tensor_copy(v_sb[:], qkv_ps[:, 640:768])

            # ---- rope on q|k (cols 0:640): out = t*C + rot(t)*S, bf16 out.
            # The rotate-half is folded into the sin-muls (strided psum reads)
            ps4 = qkv_ps[:, 0:640].rearrange("p (blk h j) -> p blk h j", blk=5, h=2)
            ropedf = work.tile([128, 640], f32, tag="ropedf")
            nc.vector.tensor_mul(ropedf[:], qkv_ps[:, 0:640], ropeCs[:])
            t2 = work.tile([128, 640], f32, tag="t2")
            t24 = t2[:].rearrange("p (blk h j) -> p blk h j", blk=5, h=2)
            rs4 = ropeSs[:].rearrange("p (blk h j) -> p blk h j", blk=5, h=2)
            nc.vector.tensor_mul(t24[:, :, 0, :], ps4[:, :, 1, :], rs4[:, :, 0, :])
            nc.vector.tensor_mul(t24[:, :, 1, :], ps4[:, :, 0, :], rs4[:, :, 1, :])
            roped = work.tile([128, 640], bf16, tag="roped")
            with nc.allow_low_precision(reason="roped q/k in bf16, matmul input"):
                nc.vector.tensor_add(roped[:], ropedf[:], t2[:])

            # ---- transpose q heads -> q_allT [128(d), (b r s)] bf16
            # (bf16 transposes are single-pass; f32 would be LOW_HIGH 2x)
            q_allT = work.tile([128, B * R * S], bf16, tag="q_allT")
            qv = q_allT[:].rearrange("p (b r s) -> p b r s", b=B, r=R)
            for r in range(R):
                tp = ps_sc.tile([128, 512], bf16, tag="sc")
                nc.tensor.transpose(
                    tp[:, 0:128], roped[:, r * 128 : (r + 1) * 128], ident_bf[:]
                )
                nc.vector.tensor_copy(
                    qv[:, :, r, :], tp[:, 0:128].rearrange("p (b s) -> p b s", b=B)
                )
            # transpose new k -> kT_sb [128(d), (b s)] bf16
            kT_sb = work.tile([128, 128], bf16, tag="kT_sb")
            tpk = ps_sc.tile([128, 512], bf16, tag="sc")
            nc.tensor.transpose(tpk[:, 0:128], roped[:, 512:640], ident_bf[:])
            nc.vector.tensor_copy(kT_sb[:], tpk[:, 0:128])

            # attention outputs per head, [128(d), (b s)] bf16, all batches
            attn_allT = [
                work.tile([128, BS], bf16, tag=f"attn{r}", name=f"attn{r}")
                for r in range(R)
            ]

            # ---- phase 2: attention per batch
            pending = []  # (b, outT_ps, recip) awaiting bc+scale

            # single psum bank holds both in-flight PV accumulators (64 cols
            # each, alternating); start=True clears has_written bits bank-wide
            # but not data, and the neighbor group is always stopped by then
            outT_duo = ps_out.tile([128, 128], f32, tag="po", name="outT_duo")

            def flush_pending():
                while pending:
                    pb, p_recip = pending.pop(0)
                    lo = (pb % 2) * 64
                    bc_ps = ps_sc.tile([128, 512], f32, tag="sc", name="bc_ps")
                    nc.tensor.matmul(
                        bc_ps[:, 0:64], ones_row[:], p_recip[:], start=True, stop=True
                    )
                    bc_sb = epool.tile([128, 64], f32, tag="bc_sb", name="bc_sb")
                    nc.vector.tensor_copy(bc_sb[:], bc_ps[:, 0:64])
                    for r in range(R):
                        nc.vector.tensor_mul(
                            attn_allT[r][:, pb * S : (pb + 1) * S],
                            outT_duo[:, lo + r * S : lo + (r + 1) * S],
                            bc_sb[:, r * S : (r + 1) * S],
                        )

            # kv stream: kk/vv interleaved in consumption order on the sync
            # ring, reusing the w_in buffers (kst rotation). kk0/vv0 have no
            # buffer dependency, so gate them on w_in slice 0 (tiny copies)
            # to keep the scheduler from hoisting their issue ahead of the
            # w_in stream; all later tiles carry natural buffer-reuse deps.
            kk_tiles = {}
            vv_tiles = {}
            for pb in range(B):
                kk = kst.tile([128, T], bf16, tag="kst", name=f"kk{pb}")
                vv = kst.tile([128, T], bf16, tag="kst", name=f"vv{pb}")
                if pb == 0:
                    nc.vector.tensor_copy(kk[0:1, 0:2], w_inT[0][0:1, 0:2])
                    nc.vector.tensor_copy(vv[0:1, 0:2], w_inT[0][0:1, 0:2])
                nc.sync.dma_start(kk[:], kT_d[pb][:])
                nc.sync.dma_start(vv[:], v_d[pb][:])
                kk_tiles[pb] = kk
                vv_tiles[pb] = vv

            for b in range(B):
                kk = kk_tiles.pop(b)
                # overwrite stale tail keys with roped new keys (gpsimd DMA:
                # a vector copy here would stall the DVE stream on kv data)
                nc.gpsimd.dma_start(
                    kk[:, OFFSET:T], kT_sb[:, b * S : (b + 1) * S]
                )
                vv = vv_tiles.pop(b)
                # patch new v tokens: chunk 31, partitions 112..127
                nc.gpsimd.dma_start(
                    vv[112:128, 31 * 128 : 32 * 128], v_sb[b * S : (b + 1) * S, :]
                )

                expS = epool.tile([128, TCH * 64], bf16, tag="expS")
                denom_ps = ps_den.tile([1, 512], f32, tag="den")
                olo = (b % 2) * 64
                # QK into 2 rotating half-banks (bufs=2: QK(b,h) reuses the
                # buffer exp(b-1,h) read, so no intra-batch ACT serialization)
                # with ONE wide [128,1024] exp per half
                for h in range(2):
                    sc = ps_sc.tile([128, 1024], f32, tag="sc")
                    for tt in range(16):
                        t = h * 16 + tt
                        nc.tensor.matmul(
                            sc[:, tt * 64 : (tt + 1) * 64],
                            kk[:, t * 128 : (t + 1) * 128],
                            q_allT[:, b * 64 : (b + 1) * 64],
                            start=True,
                            stop=True,
                        )
                    nc.scalar.activation(
                        expS[:, h * 1024 : (h + 1) * 1024], sc[:], ActExp
                    )
                    if h == 1 and b == 4:
                        # w_out: gate each slice's DMA behind a tiny memset
                        # (WAW dep) so the scheduler cannot hoist them ahead
                        # of the kv stream on the sync ring
                        for n in range(8):
                            nc.vector.memset(w_outT[n][0:1, 0:1], 0.0)
                        for n in range(8):
                            nc.sync.dma_start(w_outT[n][:], wout_d[n][:])
                # PV accumulation over all 32 chunks
                for t in range(TCH):
                    nc.tensor.matmul(
                        outT_duo[:, olo : olo + 64],
                        vv[:, t * 128 : (t + 1) * 128],
                        expS[:, t * 64 : (t + 1) * 64],
                        start=(t == 0),
                        stop=(t == TCH - 1),
                    )
                # denominator: four wide N=512 matmuls, one accumulation
                # group -> [1, 512] partial sums
                for i in range(4):
                    nc.tensor.matmul(
                        denom_ps[:],
                        ones_col[:],
                        expS[:, i * 512 : (i + 1) * 512],
                        start=(i == 0),
                        stop=(i == 3),
                    )
                # normalize the previous batch (its reciprocal is long ready)
                flush_pending()
                # tree-reduce the 8 chunk-groups: [1,512] -> [1,64]
                # (TT ops can read at most one PSUM input, so stage to SBUF)
                dred = epool.tile([1, 512], f32, tag="dred", name="dred")
                nc.vector.tensor_copy(dred[:, 0:512], denom_ps[:])
                nc.vector.tensor_add(dred[:, 0:256], dred[:, 0:256], dred[:, 256:512])
                nc.vector.tensor_add(dred[:, 0:128], dred[:, 0:128], dred[:, 128:256])
                nc.vector.tensor_add(dred[:, 0:64], dred[:, 0:64], dred[:, 64:128])
                recip = epool.tile([1, 64], bf16, tag="recip", name="recip")
                with nc.allow_low_precision(
                    reason="1/denom in bf16: 0.4% scale noise, within tolerance"
                ):
                    nc.vector.reciprocal(recip[:], dred[:, 0:64])
                pending.append((b, recip))
            flush_pending()

            # ---- phase 3: out projection, full batch [128, 4096]
            # out[bs, e_out] = sum_r attn_rT.T @ w_out[r-block, e_out-slice]
            ActCopy = mybir.ActivationFunctionType.Copy
            for n in range(8):
                po = ps_sc.tile([128, 512], f32, tag="sc")
                for r in range(R):
                    nc.tensor.matmul(
                        po[:],
                        attn_allT[r][:],
                        w_outT[n][:, r * 512 : (r + 1) * 512],
                        start=(r == 0),
                        stop=(r == R - 1),
                    )
                osb = opool.tile([128, 512], f16, tag="osb")
                # alternate the psum->sbuf drain between DVE and ACT so
                # consecutive slices overlap
                if n % 2 == 0:
                    nc.vector.tensor_copy(osb[:], po[:])
                else:
                    nc.scalar.activation(osb[:], po[:], ActCopy)
                nc.scalar.dma_start(out_d[:, n * 512 : (n + 1) * 512], osb[:])

    nc.compile()
    return nc


def _host_shards(x, w_in, w_out, k_cache, v_cache):
    """Per-core input dicts, pre-permuted for contiguous [128, N] DMAs."""
    x2 = np.ascontiguousarray(x.reshape(BS, E))
    xT_perm = (
        x2.T.reshape(KCH, 128, 128).transpose(1, 0, 2).reshape(128, KCH * 128)
    ).astype(BF16)

    # rope tables (identical on all cores); fold attn scale into q blocks
    inv_freq = 1.0 / (ROPE_BASE ** (np.arange(0, HD, 2, dtype=np.float64) / HD))
    pos = (OFFSET + np.arange(S)).astype(np.float64)
    ang = pos[:, None] * inv_freq[None, :]          # [S, 64]
    cos16 = np.cos(ang).astype(np.float32)
    sin16 = np.sin(ang).astype(np.float32)
    scale = np.float32(1.0 / np.sqrt(HD))
    C = np.zeros((128, 640), np.float32)
    Sn = np.zeros((128, 640), np.float32)
    srow = np.arange(128) % S                        # partition p=(b,s) -> s
    for blk in range(5):
        blk_scale = scale if blk < 4 else np.float32(1.0)
        C[:, blk * 128 : blk * 128 + 64] = cos16[srow] * blk_scale
        C[:, blk * 128 + 64 : blk * 128 + 128] = cos16[srow] * blk_scale
        Sn[:, blk * 128 : blk * 128 + 64] = -sin16[srow] * blk_scale
        Sn[:, blk * 128 + 64 : blk * 128 + 128] = sin16[srow] * blk_scale

    shards = []
    for g in range(NCORES):
        rows = np.concatenate(
            [
                w_in[QF * g : QF * (g + 1)],
                w_in[E + HD * g : E + HD * (g + 1)],
                w_in[E + HKV * HD + HD * g : E + HKV * HD + HD * (g + 1)],
            ],
            axis=0,
        )  # [768, 4096]
        w_inT_perm = (
            rows.T.reshape(KCH, 128, 768)
            .transpose(1, 0, 2)
            .reshape(128, KCH * 768)
        ).astype(BF16)
        w_inT_sl = np.ascontiguousarray(
            w_inT_perm.reshape(128, NSL, KPS * 768).transpose(1, 0, 2)
        )  # [NSL, 128, KPS*768]
        w_outT = np.ascontiguousarray(
            w_out[:, QF * g : QF * (g + 1)].T
        )  # [512 e_in, 4096 e_out]
        wout8 = np.empty((8, 128, R * 512), np.float32)
        for n in range(8):
            for r in range(R):
                wout8[n, :, r * 512 : (r + 1) * 512] = w_outT[
                    r * 128 : (r + 1) * 128, n * 512 : (n + 1) * 512
                ]
        kT = np.ascontiguousarray(
            k_cache[:, :, g, :].transpose(0, 2, 1)
        ).astype(BF16)  # [B, 128(d), T]
        vperm = np.ascontiguousarray(
            v_cache[:, :, g, :]
            .reshape(B, TCH, 128, HD)
            .transpose(0, 2, 1, 3)
            .reshape(B, 128, T)
        ).astype(BF16)  # [B, 128(t_in), (chunk d)]
        shards.append(
            {
                "xT": xT_perm,
                "w_inT": w_inT_sl,
                "w_outT": wout8.astype(BF16),
                "rope_c": C,
                "rope_s": Sn,
                "kT": kT,
                "vperm": vperm,
            }
        )
    return shards


def _get_nc():
    if "nc" not in _CACHED:
        _CACHED["nc"] = _build_program()
    return _CACHED["nc"]


def run_on_hw(in_maps, trace=False, **kw):
    from concourse import bass_utils

    nc = _get_nc()
    return bass_utils.run_bass_kernel_spmd(
        nc, in_maps, core_ids=list(range(NCORES)), trace=trace, **kw
    )


def kernel(x, w_in, w_out, k_cache, v_cache, offset):
    assert int(offset) == OFFSET and x.shape == (B, S, E)
    shards = _host_shards(
        np.asarray(x, np.float32),
        np.asarray(w_in, np.float32),
        np.asarray(w_out, np.float32),
        np.asarray(k_cache, np.float32),
        np.asarray(v_cache, np.float32),
    )
    res = run_on_hw(shards)
    out = np.zeros((BS, E), np.float64)
    for g in range(NCORES):
        out += np.asarray(res.results[g]["out"], np.float64)
    return out.astype(np.float32).reshape(B, S, E)
